# revision 1
# baseline (speedup 1.0000x reference)
"""Trainium2 Bass kernel for nn_MultiHeadSelfAttention_22668837388979.

Sharding: 8 cores = batch(2) x head-groups(4).  Each core handles one batch
element and 4 of the 16 heads:
  - QKV projection (bf16 matmuls, f32 accum) for its heads
  - causal ghost-softmax attention
  - row-parallel output projection partial  [2048, 1024]
Host sums the 4 head-group partials per batch element and adds Wo_b.

Ghost softmax identity used on device (avoids a max-subtraction pass):
  S = exp(s - m) / (sum exp(s - m) + g)  ==  z / (sum z + g * e^m),  z = exp(s)
so we need per-row max m only for the ghost term.  m comes from a second
(transposed-layout-free) score pass reduced on VectorE; sum z comes for free
from a ones-column appended to V in the S@V matmul.
"""

import math

import numpy as np
import ml_dtypes

EMBED = 1024
NHEAD = 16
D = 64
B = 2
S = 2048
HPC = 4          # heads per core
NCORES = 8
P = 128
NEG = -1.0e9

_prog_cache = {}


def _build_program():
    import concourse.bass as bass
    import concourse.tile as tile
    from concourse import bacc, mybir

    f32 = mybir.dt.float32
    bf16 = mybir.dt.bfloat16
    AF = mybir.ActivationFunctionType
    ALU = mybir.AluOpType
    AX = mybir.AxisListType

    nc = bacc.Bacc("TRN2", target_bir_lowering=False, debug=False)

    xT = nc.dram_tensor("xT", [EMBED, S], bf16, kind="ExternalInput").ap()
    wqkT = nc.dram_tensor("wqkT", [EMBED, 2 * HPC * D], bf16, kind="ExternalInput").ap()
    wvT = nc.dram_tensor("wvT", [EMBED, HPC * D], bf16, kind="ExternalInput").ap()
    woT = nc.dram_tensor("woT", [HPC * D, EMBED], bf16, kind="ExternalInput").ap()
    qkb = nc.dram_tensor("qkb", [P, 4], f32, kind="ExternalInput").ap()
    vb = nc.dram_tensor("vb", [1, HPC * D], bf16, kind="ExternalInput").ap()
    lng = nc.dram_tensor("lng", [P, HPC], f32, kind="ExternalInput").ap()
    idbf = nc.dram_tensor("idbf", [P, P], bf16, kind="ExternalInput").ap()
    idf = nc.dram_tensor("idf", [P, P], f32, kind="ExternalInput").ap()
    trU = nc.dram_tensor("trU", [P, P], bf16, kind="ExternalInput").ap()
    trL = nc.dram_tensor("trL", [P, P], bf16, kind="ExternalInput").ap()
    out = nc.dram_tensor("out", [S // P, 2, P, 512], bf16,
                         kind="ExternalOutput").ap()

    with tile.TileContext(nc) as tc:
        _body(tc, bass, mybir, f32, bf16, AF, ALU, AX,
              xT, wqkT, wvT, woT, qkb, vb, lng, idbf, idf, trU, trL, out)

    nc.compile()
    return nc


def _body(tc, bass, mybir, f32, bf16, AF, ALU, AX,
          xT, wqkT, wvT, woT, qkb, vb, lng, idbf, idf, trU, trL, out):
    from contextlib import ExitStack
    nc = tc.nc
    NQT = S // P                 # 16 query tiles per head
    ctx = ExitStack()

    const = ctx.enter_context(tc.tile_pool(name="const", bufs=1))
    xpool = ctx.enter_context(tc.tile_pool(name="xpool", bufs=1))
    wpool = ctx.enter_context(tc.tile_pool(name="wpool", bufs=1))
    qkt = ctx.enter_context(tc.tile_pool(name="qkt", bufs=1))
    vsb = ctx.enter_context(tc.tile_pool(name="vsb", bufs=1))
    zt = ctx.enter_context(tc.tile_pool(name="zt", bufs=34))
    mpool = ctx.enter_context(tc.tile_pool(name="mpool", bufs=2))
    spool = ctx.enter_context(tc.tile_pool(name="spool", bufs=4))
    at = ctx.enter_context(tc.tile_pool(name="at", bufs=1))
    osb = ctx.enter_context(tc.tile_pool(name="osb", bufs=2))
    dscr = ctx.enter_context(tc.tile_pool(name="dscr", bufs=2, space="DRAM"))

    uep = ctx.enter_context(tc.tile_pool(name="uep", bufs=1))
    drp = ctx.enter_context(tc.tile_pool(name="drp", bufs=2))

    ps_row = ctx.enter_context(tc.tile_pool(name="ps_row", bufs=3, space="PSUM"))
    ps_u = ctx.enter_context(tc.tile_pool(name="ps_u", bufs=2, space="PSUM"))

    def heater(n=16):
        """Dense K=128 matmul burst: re-warms the PE HAM clock gate after a
        pipeline bubble (K=64 / M=65 attention matmuls cannot un-throttle
        the PE by themselves; only sustained full-row-group activity can)."""
        hp = ps_row.tile([P, 1024], f32, tag="row", name="hp")[:, :512]
        for i in range(n):
            nc.tensor.matmul(hp, id_bf, wq_heat, start=True, stop=True)
        nc.vector.tensor_copy(heat_sink, hp[:, 0:1])

    # ---- constants / inputs to SBUF -------------------------------------
    id_bf = const.tile([P, P], bf16)
    nc.sync.dma_start(id_bf, idbf)
    id_f = const.tile([P, P], f32)
    nc.sync.dma_start(id_f, idf)
    tru_sb = const.tile([P, P], bf16)
    nc.sync.dma_start(tru_sb, trU)
    trl_sb = const.tile([P, P], bf16)
    nc.sync.dma_start(trl_sb, trL)
    qkb_sb = const.tile([P, 4], f32)
    nc.sync.dma_start(qkb_sb, qkb)
    vb_sb = const.tile([1, HPC * D], bf16)
    nc.sync.dma_start(vb_sb, vb)
    lng_sb = const.tile([P, HPC], f32)
    nc.sync.dma_start(lng_sb, lng)
    ones1 = const.tile([1, P], bf16)
    nc.vector.memset(ones1, 1.0)
    heat_sink = const.tile([P, 1], f32)
    wq_heat = const.tile([P, 512], bf16)
    nc.vector.memset(wq_heat, 0.0)

    xT_sb = xpool.tile([P, 8, S], bf16)
    wqkT_sb = wpool.tile([P, 8, 2 * HPC * D], bf16)
    wvT_sb = wpool.tile([P, 8, HPC * D], bf16)
    for e in range(8):
        nc.sync.dma_start(xT_sb[:, e, :], xT[e * P:(e + 1) * P, :])
        nc.sync.dma_start(wqkT_sb[:, e, :], wqkT[e * P:(e + 1) * P, :])
        nc.sync.dma_start(wvT_sb[:, e, :], wvT[e * P:(e + 1) * P, :])
    woT_sb = wpool.tile([P, 2, EMBED], bf16)
    nc.sync.dma_start(woT_sb, woT.rearrange("(b p) e -> p b e", p=P))

    # ---- phase 1: QKV^T projection -> QKT_sb ----------------------------
    # QKT_sb blocks: 0,1 = Q^T heads (0,1),(2,3) scaled by 1/8; 2,3 = K^T.
    QKT_sb = qkt.tile([P, 4, S], bf16)
    with nc.named_scope("qkv_proj"):
        for fb in range(4):
            for qc in range(4):
                ps = ps_row.tile([P, 1024], f32, tag="row", name="ps")
                pss = ps[:, :512]
                for e in range(8):
                    nc.tensor.matmul(
                        pss,
                        wqkT_sb[:, e, fb * P:(fb + 1) * P],
                        xT_sb[:, e, qc * 512:(qc + 1) * 512],
                        start=(e == 0), stop=(e == 7),
                    )
                nc.scalar.activation(
                    QKT_sb[:, fb, qc * 512:(qc + 1) * 512], pss,
                    AF.Identity, bias=qkb_sb[:, fb:fb + 1],
                    scale=(0.125 if fb < 2 else 1.0),
                )

    # ---- phase 2: V projection -> V_sb (with ones column) ---------------
    V_sb = vsb.tile([P, NQT, HPC, D + 1], bf16)
    nc.vector.memset(V_sb[:, :, :, D:D + 1], 1.0)
    with nc.named_scope("v_proj"):
        for st in range(NQT):
            ps = ps_row.tile([P, 1024], f32, tag="row", name="ps")
            pss = ps[:, :HPC * D]
            for e in range(8):
                nc.tensor.matmul(
                    pss,
                    xT_sb[:, e, st * P:(st + 1) * P],
                    wvT_sb[:, e, :],
                    start=(e == 0), stop=False,
                )
            nc.tensor.matmul(pss, ones1, vb_sb, start=False, stop=True)
            nc.scalar.activation(
                V_sb[:, st, :, 0:D],
                pss.rearrange("p (h d) -> p h d", h=HPC),
                AF.Copy,
            )

    heater()

    # ---- phase 3+4: attention (head pairs) + per-pair output projection --
    # Heads are processed in pairs (0,1) and (2,3).  Within a pair the two
    # heads' score matmuls use PE row-groups (0,0)/(64,0) and run
    # concurrently in the array.  All z_T rows of a (head, half) are
    # materialized in SBUF so the S@V accumulations run as dense matmul
    # bursts (keeps the PE HAM un-throttled).
    AT_sb = at.tile([P, 2, S], bf16)   # A^T stacked: partitions = head%2*64+d

    for pair in range(2):
        m_cols = {}
        with nc.named_scope(f"maxpass_p{pair}"):
            for hh in range(2):
                m_cols[hh] = mpool.tile([P, NQT], f32, tag=f"mcol{hh}",
                                        name="m_col")
            for qt in range(NQT):
                W = (qt + 1) * P
                for hh in range(2):
                    h = 2 * pair + hh
                    poff = D * hh
                    ntile = (W + 1023) // 1024
                    for ti in range(ntile):
                        ts0 = ti * 1024
                        tw = min(1024, W - ts0)
                        pm = ps_row.tile([P, 1024], f32, tag="row",
                                         name="pm")[:, :tw]
                        for cs in range(0, tw, 512):
                            cw = min(512, tw - cs)
                            has_diag = (ts0 + cs + cw == W)
                            if not has_diag:
                                nc.tensor.matmul(
                                    pm[:, cs:cs + cw],
                                    QKT_sb[poff:poff + D, pair,
                                           qt * P:(qt + 1) * P],
                                    QKT_sb[poff:poff + D, 2 + pair,
                                           ts0 + cs:ts0 + cs + cw],
                                    start=True, stop=True,
                                )
                                continue
                            if cw > P:
                                nc.tensor.matmul(
                                    pm[:, cs:cs + cw - P],
                                    QKT_sb[poff:poff + D, pair,
                                           qt * P:(qt + 1) * P],
                                    QKT_sb[poff:poff + D, 2 + pair,
                                           ts0 + cs:ts0 + cs + cw - P],
                                    start=True, stop=True,
                                )
                            nc.tensor.matmul(
                                pm[:, cs + cw - P:cs + cw], id_bf, tru_sb,
                                start=True, stop=False, skip_group_check=True,
                            )
                            nc.tensor.matmul(
                                pm[:, cs + cw - P:cs + cw],
                                QKT_sb[poff:poff + D, pair,
                                       qt * P:(qt + 1) * P],
                                QKT_sb[poff:poff + D, 2 + pair,
                                       ts0 + cs + cw - P:ts0 + cs + cw],
                                start=False, stop=True, skip_group_check=True,
                            )
                        if ti == 0:
                            nc.vector.reduce_max(
                                m_cols[hh][:, qt:qt + 1], pm, axis=AX.X)
                        else:
                            mtmp = spool.tile([P, 1], f32, tag="mtmp")
                            nc.vector.reduce_max(mtmp, pm, axis=AX.X)
                            nc.vector.tensor_tensor(
                                m_cols[hh][:, qt:qt + 1],
                                m_cols[hh][:, qt:qt + 1], mtmp, ALU.max,
                            )

        heater()

        # --- transposed scores -> exp -> z_T (materialized per half) -> S@V
        Ue = {}
        for hh in range(2):
            Ue[hh] = uep.tile([D + 1, S], f32, tag=f"ue{hh}", name="Ue")
        with nc.named_scope(f"sv_p{pair}"):
            for H in range(2):
                if H == 1:
                    heater(8)
                q0 = H * 1024
                kmax = 8 if H == 0 else 16
                zrows = {}
                for hh in range(2):
                    h = 2 * pair + hh
                    poff = D * hh
                    for kj in range(kmax):
                        rs = max(kj * P, q0)
                        rw = q0 + 1024 - rs
                        pt = ps_row.tile([P, 1024], f32, tag="row",
                                         name="pt")[:, :rw]
                        row_diag = (kj * P >= q0)
                        nck = (rw + 511) // 512
                        for ci in range(nck):
                            cw = min(512, rw - ci * 512)
                            if row_diag and ci == 0:
                                nc.tensor.matmul(
                                    pt[:, 0:P], id_bf, trl_sb,
                                    start=True, stop=False,
                                    skip_group_check=True,
                                )
                                nc.tensor.matmul(
                                    pt[:, 0:P],
                                    QKT_sb[poff:poff + D, 2 + pair,
                                           kj * P:(kj + 1) * P],
                                    QKT_sb[poff:poff + D, pair, rs:rs + P],
                                    start=False, stop=True,
                                    skip_group_check=True,
                                )
                                if cw > P:
                                    nc.tensor.matmul(
                                        pt[:, P:cw],
                                        QKT_sb[poff:poff + D, 2 + pair,
                                               kj * P:(kj + 1) * P],
                                        QKT_sb[poff:poff + D, pair,
                                               rs + P:rs + cw],
                                        start=True, stop=True,
                                    )
                            else:
                                nc.tensor.matmul(
                                    pt[:, ci * 512:ci * 512 + cw],
                                    QKT_sb[poff:poff + D, 2 + pair,
                                           kj * P:(kj + 1) * P],
                                    QKT_sb[poff:poff + D, pair,
                                           rs + ci * 512:rs + ci * 512 + cw],
                                    start=True, stop=True,
                                )
                        zr = zt.tile([P, 1024], bf16, tag="zrow",
                                     name="zr")[:, :rw]
                        nc.scalar.activation(zr, pt, AF.Exp)
                        zrows[(hh, kj)] = (zr, rs)
                # dense S@V bursts, one U quarter at a time
                for hh in range(2):
                    h = 2 * pair + hh
                    for qq in (2 * H, 2 * H + 1):
                        nkj = min(kmax, (qq + 1) * 4)
                        Uq = ps_u.tile([D + 1, 512], f32, tag="u", name="Uq")
                        for kj in range(nkj):
                            zr, rs = zrows[(hh, kj)]
                            a = max(rs, qq * 512)
                            w = (qq + 1) * 512 - a
                            nc.tensor.matmul(
                                Uq[:, a - qq * 512:a - qq * 512 + w],
                                V_sb[:, kj, h, :],
                                zr[:, a - rs:a - rs + w],
                                start=(kj == 0), stop=(kj == nkj - 1),
                            )
                        nc.vector.tensor_copy(
                            Ue[hh][:, qq * 512:(qq + 1) * 512], Uq)

        for hh in range(2):
            h = 2 * pair + hh
            with nc.named_scope(f"stats_h{h}"):
                _stats_and_scale(tc, nc, bass, mybir, f32, AF, ALU,
                                 spool, drp, ps_u, dscr, Ue[hh], m_cols[hh],
                                 lng_sb, id_f, AT_sb, h)

    heater()

    # ---- output projection (all heads) -> tiled bf16 output --------------
    with nc.named_scope("out_proj"):
        for qt in range(NQT):
            for ec in range(2):
                po = ps_row.tile([P, 1024], f32, tag="row",
                                 name="po")[:, :512]
                for b in range(2):
                    nc.tensor.matmul(
                        po,
                        AT_sb[:, b, qt * P:(qt + 1) * P],
                        woT_sb[:, b, ec * 512:(ec + 1) * 512],
                        start=(b == 0), stop=(b == 1),
                    )
                ot = osb.tile([P, 512], bf16, tag="ot")
                if ec == 0:
                    nc.vector.tensor_copy(ot, po)
                else:
                    nc.scalar.activation(ot, po, AF.Copy)
                nc.sync.dma_start(out[qt, ec], ot)


    ctx.close()


def _stats_and_scale(tc, nc, bass, mybir, f32, AF, ALU, spool, drp, ps_u,
                     dscr, Ue_sb, m_col, lng_sb, id_f, AT_sb, h):
    """denom = sum z + g*e^m per row; A^T[head] = U^T * (1/denom)."""
    P = 128
    # transpose sum-z row [1, 2048] -> column [128, 16] in psum
    szcol = ps_u.tile([P, 16], f32, tag="u", name="szcol")
    for c in range(16):
        nc.tensor.transpose(
            szcol[:, c:c + 1], Ue_sb[D:D + 1, c * P:(c + 1) * P],
            id_f[D:D + 1, D:D + 1],
        )
    ghost = spool.tile([P, 16], f32, tag="ghost", name="ghost")
    nc.scalar.activation(
        ghost, m_col, AF.Exp, bias=lng_sb[:, h:h + 1], scale=1.0,
    )
    denom = spool.tile([P, 16], f32, tag="denom", name="denom")
    nc.vector.tensor_tensor(denom, szcol, ghost, ALU.add)
    dcol = spool.tile([P, 16], f32, tag="dcol", name="dcol")
    nc.vector.reciprocal(dcol, denom)
    dT = ps_u.tile([16, P], f32, tag="u", name="dT")
    nc.tensor.transpose(dT, dcol, id_f)
    dT_sb = spool.tile([16, P], f32, tag="dtsb", name="dT_sb")
    nc.vector.tensor_copy(dT_sb, dT)
    # reshape [16,128] -> [1,2048] via DRAM bounce (partition -> free merge)
    dram_t = dscr.tile([16, P], f32, tag="dbounce", name="dram_t")
    nc.sync.dma_start(dram_t, dT_sb)
    # replicate the reciprocal-denominator row across 64 partitions via DMA
    drow = drp.tile([D, S], f32, tag="drow", name="drow")
    nc.sync.dma_start(
        drow, dram_t.rearrange("c w -> (c w)")[None, :].to_broadcast((D, S)),
    )
    # A^T[head] = U^T * drow, cast to bf16
    nc.vector.tensor_tensor(
        AT_sb[D * (h % 2):D * (h % 2) + D, h // 2, :],
        Ue_sb[0:D, :],
        drow,
        ALU.mult,
    )


def _host_inputs(inputs, Wqkv_w, Wqkv_b, Wo_w, ghost):
    """Build the 8 per-core input maps."""
    bf = ml_dtypes.bfloat16
    idf = np.eye(P, dtype=np.float32)
    idbf = np.eye(P, dtype=bf)
    trUm = (np.triu(np.ones((P, P), np.float32), 1) * NEG).astype(bf)
    trLm = (np.tril(np.ones((P, P), np.float32), -1) * NEG).astype(bf)
    in_maps = []
    for core in range(NCORES):
        b = core // 4
        g = core % 4
        r0 = g * HPC * D
        r1 = (g + 1) * HPC * D
        Wq = Wqkv_w[r0:r1]                      # [256, 1024]
        Wk = Wqkv_w[NHEAD * D + r0:NHEAD * D + r1]
        Wv = Wqkv_w[2 * NHEAD * D + r0:2 * NHEAD * D + r1]
        qk_bias = np.concatenate([Wqkv_b[r0:r1] / 8.0,
                                  Wqkv_b[NHEAD * D + r0:NHEAD * D + r1]])
        g_h = np.maximum(ghost[g * HPC:(g + 1) * HPC].astype(np.float64), 1e-38)
        in_maps.append({
            "xT": np.ascontiguousarray(inputs[b].T).astype(bf),
            "wqkT": np.ascontiguousarray(np.concatenate([Wq, Wk], 0).T).astype(bf),
            "wvT": np.ascontiguousarray(Wv.T).astype(bf),
            "woT": np.ascontiguousarray(Wo_w[:, r0:r1].T).astype(bf),
            "qkb": np.ascontiguousarray(
                qk_bias.reshape(4, P).T).astype(np.float32),
            "vb": Wqkv_b[2 * NHEAD * D + r0:2 * NHEAD * D + r1][None, :].astype(bf),
            "lng": np.tile(np.log(g_h).astype(np.float32)[None, :], (P, 1)),
            "idbf": idbf, "idf": idf, "trU": trUm, "trL": trLm,
        })
    return in_maps


def kernel(inputs, Wqkv_w, Wqkv_b, Wo_w, Wo_b, ghost, _trace=False, _cores=NCORES):
    inputs = np.asarray(inputs, dtype=np.float32)
    Wqkv_w = np.asarray(Wqkv_w, dtype=np.float32)
    Wqkv_b = np.asarray(Wqkv_b, dtype=np.float32)
    Wo_w = np.asarray(Wo_w, dtype=np.float32)
    Wo_b = np.asarray(Wo_b, dtype=np.float32)
    ghost = np.asarray(ghost, dtype=np.float32)

    from concourse import bass_utils

    if "nc" not in _prog_cache:
        _prog_cache["nc"] = _build_program()
    nc = _prog_cache["nc"]

    in_maps = _host_inputs(inputs, Wqkv_w, Wqkv_b, Wo_w, ghost)
    res = bass_utils.run_bass_kernel_spmd(
        nc, in_maps[:_cores], core_ids=list(range(_cores)), trace=_trace,
    )
    full = np.zeros((B, S, EMBED), np.float32)
    for core in range(_cores):
        o = res.results[core]["out"].astype(np.float32)   # [16, 2, 128, 512]
        full[core // 4] += o.transpose(0, 2, 1, 3).reshape(S, EMBED)
    full += Wo_b[None, None, :]
    if _trace:
        _prog_cache["last_results"] = res
    return full

